# revision 1
# baseline (speedup 1.0000x reference)
"""Trainium2 Bass kernel for nn_AutoEncoder_64854006170336.

Per-joint-embedding transformer encoder (B=1024, A=25 tokens, D=512, H=8, L=6).

Strategy:
- Data-parallel over batch: 8 cores x 128 batches each. No collectives.
- bf16 matmul operands, fp32 PSUM accumulation.
- Pack-contiguous token order: packs of G=4 batches; token (b=4g+j, pos a)
  lives at column 112*g + 25*j + a of the D-major tensors (112 = padded pack
  stride for DMA-transpose's mult-16 partition rule; cols 100..111 of each
  pack are dead).
- D-major tensors hT/kT/qT/oT [128, 4, 3584] (row d = ch*128 + p) feed all
  matmuls with contiguous stationary operands; token-major pack tiles
  [112, 512] carry residual/LayerNorm and are DMA-transposed back to D-major.
- q projection is weights-stationary (out = qT chunk directly); its moving
  operand is a strided position-gather AP, its evacuation a strided DVE write.
- Attention per (head, pack): scores via one 64x100x100 matmul, exp on ACT
  (scale=1/8 folds sqrt(depth)), cross-batch garbage zeroed by a mask
  multiply, softmax denominator via a ones-column matmul, normalization
  folded into the AV-psum evacuation.
- LayerNorm affine (g, b) folded host-side into next-layer projection
  weights/biases; residual state is xg = LNraw(p) * g (bf16) + biases carried
  in the matmuls (ACT per-partition bias or ones-row bias matmuls).
"""

from contextlib import ExitStack

import os

import numpy as np
import ml_dtypes

import concourse.bass as bass
import concourse.mybir as mybir
import concourse.tile as tile
from concourse import bacc
from concourse.bass_utils import run_bass_kernel_spmd

BF = ml_dtypes.bfloat16
bf16 = mybir.dt.bfloat16
f32 = mybir.dt.float32
AF = mybir.ActivationFunctionType
ALU = mybir.AluOpType

B, J, DI, D, H, L = 1024, 24, 64, 512, 8, 6
A = J + 1            # 25 tokens
NCORES = 8
BC = B // NCORES     # 128 batches/core
G = 4                # batches per pack
NPACK = BC // G      # 32
PT = G * A           # 100 live tokens per pack
PTP = 112            # padded pack stride (mult of 16)
TW = NPACK * PTP     # 3584 D-major token columns
DEPTH = D // H       # 64
NCH = 4
LN_EPS = 1e-5
CAT = 4              # packs per score-psum concat
TOK_TILES = [(i * 512, 512) for i in range(7)]
CATS = [(g0, min(CAT, NPACK - g0)) for g0 in range(0, NPACK, CAT)]

_compiled = None
SIM_SAFE = os.environ.get("BASS_SIM", "0") == "1"


def _ap(tensor_ap, extra_offset, dims):
    return bass.AP(tensor=tensor_ap.tensor, offset=tensor_ap.offset + extra_offset,
                   ap=dims)


def _bcast_mid(ap, n):
    return bass.AP(tensor=ap.tensor, offset=ap.offset,
                   ap=[ap.ap[0], [0, n], *ap.ap[1:]])


def _bcast_last(ap, n):
    return bass.AP(tensor=ap.tensor, offset=ap.offset, ap=[*ap.ap, [0, n]])


class _Kern:
    def __init__(self):
        self.nc = bacc.Bacc(None, target_bir_lowering=False)
        nc = self.nc
        self.x_t = nc.dram_tensor("x_t", [DI, J, BC], bf16, kind="ExternalInput")
        self.cls_row = nc.dram_tensor("cls_row", [1, D], bf16, kind="ExternalInput")
        self.We_t = nc.dram_tensor("We_t", [J, DI, D], bf16, kind="ExternalInput")
        self.be_t = nc.dram_tensor("be_t", [J, D], bf16, kind="ExternalInput")
        self.Wk_t = nc.dram_tensor("Wk_t", [L, 128, NCH, D], bf16, kind="ExternalInput")
        self.Wv_t = nc.dram_tensor("Wv_t", [L, 128, NCH, D], bf16, kind="ExternalInput")
        self.Wo_t = nc.dram_tensor("Wo_t", [L, 128, NCH, D], bf16, kind="ExternalInput")
        self.Wq_t = nc.dram_tensor(
            "Wq_t", [L, A, 128, NCH, D], bf16, kind="ExternalInput"
        )
        self.bk_t = nc.dram_tensor("bk_t", [L, 128, NCH], f32, kind="ExternalInput")
        self.bq_t = nc.dram_tensor("bq_t", [L, A, 128, NCH], f32, kind="ExternalInput")
        self.bo_t = nc.dram_tensor("bo_t", [L, D], bf16, kind="ExternalInput")
        self.g_t = nc.dram_tensor("g_t", [L - 1, D], bf16, kind="ExternalInput")
        self.g5_row = nc.dram_tensor("g5_row", [1, D], f32, kind="ExternalInput")
        self.b5_row = nc.dram_tensor("b5_row", [1, D], f32, kind="ExternalInput")
        self.MU_t = nc.dram_tensor("MU_t", [G + 1, PT], bf16, kind="ExternalInput")
        self.MV_t = nc.dram_tensor("MV_t", [G + 1, CAT * PT], bf16, kind="ExternalInput")
        self.out_t = nc.dram_tensor("out", [BC, A, D], f32, kind="ExternalOutput")

    def build(self):
        nc = self.nc
        with ExitStack() as ctx:
            tc = ctx.enter_context(tile.TileContext(nc))
            p = lambda name, bufs, space="SBUF": ctx.enter_context(
                tc.tile_pool(name=name, bufs=bufs, space=space)
            )
            self.big = p("big", 1)
            self.xgp = p("xgp", NPACK)
            self.stage = p("stage", 2)
            self.vstore = p("vstore", 4)
            self.otokp = p("otok", 2)
            self.oTpool = p("oTpool", NPACK)
            self.xtp = p("xtp", 8)
            self.wts = p("wts", 1)
            self.wkp = p("wkp", 2)
            self.wqp = p("wqp", 2)
            self.rows = p("rows", 2)
            self.emws = p("emws", 2)
            self.stats = p("stats", 4)
            self.spool = p("spool", 3)
            self.ppool = p("ppool", 3)
            self.fpool = p("fpool", 2)
            self.edp = p("edram", 1, "DRAM")
            self.psA = p("psA", 2, "PSUM")
            self.psS = p("psS", 2, "PSUM")
            self.psV = p("psV", 2, "PSUM")
            self.psD = p("psD", 2, "PSUM")
            self._consts()
            self._embedding()
            for l in range(L):
                self._layer(l)
        nc.compile()
        return nc

    def _consts(self):
        nc, big = self.nc, self.big
        self.hT = big.tile([128, NCH, TW], bf16, tag="hT")
        self.kT = big.tile([128, NCH, TW], bf16, tag="kT")
        self.qT = big.tile([128, NCH, TW], bf16, tag="qT")
        self.oTp = [None] * NPACK
        self.MU = big.tile([G + 1, PT], bf16, tag="MU")
        self.MV = big.tile([G + 1, CAT * PT], bf16, tag="MV")
        self.ones_row = big.tile([1, 128], bf16, tag="ones_row")
        self.ones_col = big.tile([PT, 1], bf16, tag="ones_col")
        self.eps_t = big.tile([128, 1], f32, tag="eps")
        self.g5b = big.tile([128, D], f32, tag="g5b")
        self.b5b = big.tile([128, D], f32, tag="b5b")
        nc.scalar.dma_start(out=self.MU[:], in_=self.MU_t[:])
        nc.scalar.dma_start(out=self.MV[:], in_=self.MV_t[:])
        nc.vector.memset(self.ones_row[:], 1.0)
        nc.vector.memset(self.ones_col[:], 1.0)
        nc.vector.memset(self.eps_t[:], LN_EPS)
        nc.scalar.dma_start(out=self.g5b[:], in_=self.g5_row[:].to_broadcast((128, D)))
        nc.scalar.dma_start(out=self.b5b[:], in_=self.b5_row[:].to_broadcast((128, D)))

    def _pos_ap(self, big3, kc, a):
        """Moving-operand AP: [128, NPACK, G] = cols 112g + 25j + a of chunk kc."""
        base = big3[:, kc, :]
        return _ap(base, a, [base.ap[0], [PTP, NPACK], [A, G]])

    def _pos_out_ap(self, big3, a):
        """[128, NCH, NPACK, G] strided evac target across all chunks."""
        base = big3[:]
        return _ap(base, a, [base.ap[0], [TW, NCH], [PTP, NPACK], [A, G]])

    def _head_win(self, tens, h, g):
        return tens[(h % 2) * 64 : (h % 2) * 64 + 64, h // 2,
                    g * PTP : g * PTP + PT]

    def _embedding(self):
        nc = self.nc
        e_dram = self.edp.tile([BC, A, D], bf16, tag="edram")
        cls_sb = self.stage.tile([BC, D], bf16, tag="xhat", name="cls_sb")
        nc.scalar.dma_start(out=cls_sb[:], in_=self.cls_row[:].to_broadcast((BC, D)))
        nc.sync.dma_start(out=e_dram[:, 0, :], in_=cls_sb[:])
        for j in range(J):
            xj = self.emws.tile([DI, BC], bf16, tag="xj")
            nc.scalar.dma_start(out=xj[:], in_=self.x_t[:, j, :])
            wej = self.emws.tile([DI, D], bf16, tag="wej")
            nc.scalar.dma_start(out=wej[:], in_=self.We_t[j])
            bej = self.rows.tile([1, D], bf16, tag="brow")
            nc.scalar.dma_start(out=bej[:], in_=self.be_t[j : j + 1, :])
            ps = self.psA.tile([128, D], f32, tag="pp")
            nc.tensor.matmul(ps[:], xj[:], wej[:], start=True, stop=False)
            nc.tensor.matmul(ps[:], self.ones_row[:], bej[:], start=False, stop=True)
            ej = self.stage.tile([BC, D], bf16, tag="xhat", name="ej")
            nc.vector.tensor_copy(ej[:], ps[:])
            nc.sync.dma_start(out=e_dram[:, j + 1, :], in_=ej[:])
        self.xg = [None] * NPACK
        for g in range(NPACK):
            xg0 = self.xgp.tile([PTP, D], bf16, tag="xg")
            if SIM_SAFE:
                nc.vector.memset(xg0[96:PTP, :], 0.0)
            nc.sync.dma_start(out=xg0[:PT, :], in_=e_dram[G * g : G * g + G, :, :])
            nc.sync.dma_start(
                out=self.hT[:, :, g * PTP : (g + 1) * PTP], in_=xg0[:], transpose=True
            )
            self.xg[g] = xg0

    def _layer_weights(self, l):
        nc = self.nc
        wk = self.wkp.tile([128, NCH, D], bf16, tag="wk")
        nc.scalar.dma_start(out=wk[:], in_=self.Wk_t[l])
        wv = self.wts.tile([128, NCH, D], bf16, tag="wv")
        nc.scalar.dma_start(out=wv[:], in_=self.Wv_t[l])
        wo = self.wts.tile([128, NCH, D], bf16, tag="wo")
        nc.scalar.dma_start(out=wo[:], in_=self.Wo_t[l])
        bk = self.stats.tile([128, NCH], f32, tag="bk")
        nc.scalar.dma_start(out=bk[:], in_=self.bk_t[l])
        bo_row = self.rows.tile([1, D], bf16, tag="borow")
        nc.scalar.dma_start(out=bo_row[:], in_=self.bo_t[l : l + 1, :])
        glb = None
        if l < L - 1:
            glb = self.stage.tile([128, D], bf16, tag="glb")
            nc.gpsimd.dma_start(
                out=glb[:], in_=self.g_t[l : l + 1, :].to_broadcast((128, D))
            )
        return wk, wv, wo, bk, bo_row, glb

    def _kproj(self, wk, bk):
        nc = self.nc
        for oc in range(NCH):
            for (c0, cn) in TOK_TILES:
                ps = self.psA.tile([128, D], f32, tag="pp")
                for kc in range(NCH):
                    nc.tensor.matmul(
                        ps[:, :cn],
                        wk[:, kc, oc * 128 : (oc + 1) * 128],
                        self.hT[:, kc, c0 : c0 + cn],
                        start=(kc == 0),
                        stop=(kc == NCH - 1),
                    )
                nc.scalar.activation(
                    self.kT[:, oc, c0 : c0 + cn], ps[:, :cn],
                    AF.Identity, bias=bk[:, oc : oc + 1], scale=1.0,
                )

    def _qproj(self, l):
        nc = self.nc
        for a in range(A):
            wq = self.wqp.tile([128, NCH, D], bf16, tag="wq")
            nc.scalar.dma_start(out=wq[:], in_=self.Wq_t[l, a])
            bq = self.stats.tile([128, NCH], f32, tag="bq")
            nc.scalar.dma_start(out=bq[:], in_=self.bq_t[l, a])
            ps = self.psA.tile([128, NCH, 128], f32, tag="pp")
            for oc in range(NCH):
                for kc in range(NCH):
                    nc.tensor.matmul(
                        ps[:, oc, :],
                        wq[:, kc, oc * 128 : (oc + 1) * 128],
                        self._pos_ap(self.hT, kc, a),
                        start=(kc == 0),
                        stop=(kc == NCH - 1),
                    )
            nc.vector.tensor_tensor(
                self._pos_out_ap(self.qT, a),
                ps[:],
                _bcast_last(bq[:], 128),
                ALU.add,
            )

    def _vproj(self, wv):
        nc = self.nc
        v_sb = [None] * NPACK
        for g in range(NPACK):
            ps = self.psA.tile([128, D], f32, tag="pp")
            for kc in range(NCH):
                nc.tensor.matmul(
                    ps[:PT, :],
                    self.hT[:, kc, g * PTP : g * PTP + PT],
                    wv[:, kc, :],
                    start=(kc == 0),
                    stop=(kc == NCH - 1),
                )
            vt = self.vstore.tile([PT, D], bf16, tag="v")
            nc.scalar.copy(vt[:], ps[:PT, :])
            v_sb[g] = vt
        return v_sb

    def _attention(self, v_sb):
        nc = self.nc
        for (g0, ng) in CATS:
            xts = []
            for hp in range(H // 2):
                sc2 = [self.psS.tile([PT, CAT * PT], f32, tag="sc", name=f"sc{u}") for u in range(2)]
                for u in range(2):
                    nc.tensor.matmul(
                        sc2[u][:, : ng * PT], self.MU[:], self.MV[:, : ng * PT],
                        start=True, stop=False,
                    )
                for i in range(ng):
                    for u in range(2):
                        h = 2 * hp + u
                        nc.tensor.matmul(
                            sc2[u][:, i * PT : (i + 1) * PT],
                            self._head_win(self.kT, h, g0 + i),
                            self._head_win(self.qT, h, g0 + i),
                            start=False, stop=(i == ng - 1),
                        )
                for u in range(2):
                    xt = self.xtp.tile([PT, CAT * PT], bf16, tag="xt")
                    nc.scalar.activation(
                        xt[:, : ng * PT], sc2[u][:, : ng * PT], AF.Exp,
                        scale=1.0 / 8.0,
                    )
                    xts.append(xt)
            for i in range(ng):
                self._av(xts, v_sb, g0, i)

    def _av(self, xts, v_sb, g0, i):
        nc = self.nc
        g = g0 + i
        avps = self.psV.tile([PT, D], f32, tag="av")
        sps = self.psD.tile([PT, H], f32, tag="s")
        for h in range(H):
            xsl = xts[h][:, i * PT : (i + 1) * PT]
            nc.tensor.matmul(
                avps[:, h * DEPTH : (h + 1) * DEPTH],
                xsl, v_sb[g][:, h * DEPTH : (h + 1) * DEPTH],
                start=True, stop=True,
            )
            nc.tensor.matmul(
                sps[:, h : h + 1], xsl, self.ones_col[:], start=True, stop=True
            )
        rec = self.spool.tile([PT, H], f32, tag="rec")
        nc.vector.reciprocal(rec[:], sps[:])
        ot = self.otokp.tile([PTP, D], bf16, tag="otok")
        if SIM_SAFE:
            nc.vector.memset(ot[96:PTP, :], 0.0)
        nc.vector.tensor_tensor(
            ot[:PT, :].rearrange("p (h e) -> p h e", h=H),
            avps[:].rearrange("p (h e) -> p h e", h=H),
            _bcast_last(rec[:], DEPTH),
            ALU.mult,
        )
        oTg = self.oTpool.tile([128, NCH, PTP], bf16, tag="oT")
        nc.sync.dma_start(out=oTg[:], in_=ot[:], transpose=True)
        self.oTp[g] = oTg

    def _opack(self, lctx, g):
        l, wo, bo_row, glb = lctx
        nc = self.nc
        if True:
            ps = self.psA.tile([128, D], f32, tag="pp")
            for kc in range(NCH):
                nc.tensor.matmul(
                    ps[:PT, :],
                    self.oTp[g][:, kc, :PT],
                    wo[:, kc, :],
                    start=(kc == 0), stop=False,
                )
            nc.tensor.matmul(
                ps[:PT, :], self.ones_row[:, :PT], bo_row[:], start=False, stop=True
            )
            pt = self.ppool.tile([PT, D], f32, tag="p")
            nc.vector.tensor_add(pt[:], ps[:PT, :], self.xg[g][:PT, :])
            st6 = self.stats.tile([PT, 6], f32, tag="st6")
            nc.vector.bn_stats(st6[:], pt[:])
            mv = self.stats.tile([PT, 2], f32, tag="mv")
            nc.vector.bn_aggr(mv[:], st6[:])
            std = self.stats.tile([PT, 1], f32, tag="std")
            nc.scalar.activation(std[:], mv[:, 1:2], AF.Sqrt, bias=self.eps_t[:PT, :])
            rstd = self.stats.tile([PT, 1], f32, tag="rstd")
            nc.vector.reciprocal(rstd[:], std[:])
            nmr = self.stats.tile([PT, 1], f32, tag="nmr")
            nc.vector.tensor_scalar(
                nmr[:], mv[:, 0:1], rstd[:], -1.0, ALU.mult, ALU.mult
            )
            if l < L - 1:
                xh = self.stage.tile([PTP, D], bf16, tag="xhat")
                if SIM_SAFE:
                    nc.vector.memset(xh[96:PTP, :], 0.0)
                nc.scalar.activation(
                    xh[:PT, :], pt[:], AF.Identity, bias=nmr[:], scale=rstd[:]
                )
                nc.sync.dma_start(
                    out=self.hT[:, :, g * PTP : (g + 1) * PTP], in_=xh[:],
                    transpose=True,
                )
                xgn = self.xgp.tile([PTP, D], bf16, tag="xg")
                nc.gpsimd.tensor_mul(xgn[:PT, :], xh[:PT, :], glb[:PT, :])
                self.xg[g] = xgn
            else:
                of = self.fpool.tile([PT, D], f32, tag="of")
                nc.scalar.activation(
                    of[:], pt[:], AF.Identity, bias=nmr[:], scale=rstd[:]
                )
                nc.vector.tensor_mul(of[:], of[:], self.g5b[:PT, :])
                nc.vector.tensor_add(of[:], of[:], self.b5b[:PT, :])
                nc.sync.dma_start(
                    out=self.out_t[G * g : G * g + G, :, :], in_=of[:]
                )

    def _layer(self, l):
        wk, wv, wo, bk, bo_row, glb = self._layer_weights(l)
        self._kproj(wk, bk)
        self._qproj(l)
        v_sb = self._vproj(wv)
        self._attention(v_sb)
        for g in range(NPACK):
            self._opack((l, wo, bo_row, glb), g)


def _build():
    return _Kern().build()


def _prep_inputs(inputs):
    """Host-side fold + layout prep. Returns (shared dict, per-core x list)."""
    f = lambda v: np.asarray(v, dtype=np.float64)
    x = np.asarray(inputs["x"], dtype=np.float32)
    We, be = f(inputs["We"]), f(inputs["be"])
    cls_token = f(inputs["cls_token"])
    Wk, bk = f(inputs["Wk"]), f(inputs["bk"])
    Wv, bv = f(inputs["Wv"]), f(inputs["bv"])
    Wq, bq = f(inputs["Wq"]), f(inputs["bq"])
    Wo, bo = f(inputs["Wo"]), f(inputs["bo"])
    ln_g, ln_b = f(inputs["ln_g"]), f(inputs["ln_b"])

    def chunk_w(w):  # [512, 512] -> [128, 4, 512]
        return np.ascontiguousarray(
            w.reshape(NCH, 128, D).transpose(1, 0, 2)
        ).astype(BF)

    def chunk_b(b):  # [512] -> [128, 4]
        return np.ascontiguousarray(b.reshape(NCH, 128).T).astype(np.float32)

    Wk_t = np.zeros((L, 128, NCH, D), BF)
    Wv_t = np.zeros((L, 128, NCH, D), BF)
    Wo_t = np.zeros((L, 128, NCH, D), BF)
    Wq_t = np.zeros((L, A, 128, NCH, D), BF)
    bk_t = np.zeros((L, 128, NCH), np.float32)
    bq_t = np.zeros((L, A, 128, NCH), np.float32)
    bo_t = np.zeros((L, D), BF)
    for l in range(L):
        gf = ln_g[l - 1] if l > 0 else np.ones(D)
        bf_ = ln_b[l - 1] if l > 0 else np.zeros(D)
        bv_f = bf_ @ Wv[l] + bv[l]
        Wk_t[l] = chunk_w(gf[:, None] * Wk[l])
        Wv_t[l] = chunk_w(gf[:, None] * Wv[l])
        Wo_t[l] = chunk_w(Wo[l])
        bk_t[l] = chunk_b(bf_ @ Wk[l] + bk[l])
        bo_t[l] = (bv_f @ Wo[l] + bo[l] + bf_).astype(BF)
        for a in range(A):
            Wq_t[l, a] = chunk_w(gf[:, None] * Wq[l, a])
            bq_t[l, a] = chunk_b(bf_ @ Wq[l, a] + bq[l, a])

    MB = 400.0  # exp(-400/8) == 0 in bf16; diag contributions cancel exactly
    MU = np.zeros((G + 1, PT), BF)
    MV = np.zeros((G + 1, CAT * PT), BF)
    MU[0, :] = 1.0
    MV[0, :] = -MB
    for j in range(G):
        MU[1 + j, j * A : (j + 1) * A] = 1.0
        for i in range(CAT):
            MV[1 + j, i * PT + j * A : i * PT + (j + 1) * A] = MB

    shared = {
        "cls_row": cls_token.reshape(1, D).astype(BF),
        "We_t": We.astype(BF),
        "be_t": be.astype(BF),
        "Wk_t": Wk_t, "Wv_t": Wv_t, "Wo_t": Wo_t, "Wq_t": Wq_t,
        "bk_t": bk_t, "bq_t": bq_t, "bo_t": bo_t,
        "g_t": ln_g[: L - 1].astype(BF),
        "g5_row": ln_g[L - 1].reshape(1, D).astype(np.float32),
        "b5_row": ln_b[L - 1].reshape(1, D).astype(np.float32),
        "MU_t": MU, "MV_t": MV,
    }
    x_cores = []
    for c in range(NCORES):
        xc = x[c * BC : (c + 1) * BC]            # [128, 24, 64]
        x_cores.append(np.ascontiguousarray(xc.transpose(2, 1, 0)).astype(BF))
    return shared, x_cores


def kernel(**inputs) -> np.ndarray:
    global _compiled
    if _compiled is None:
        _compiled = _build()
    nc = _compiled
    shared, x_cores = _prep_inputs(inputs)
    in_maps = [{**shared, "x_t": x_cores[c]} for c in range(NCORES)]
    res = run_bass_kernel_spmd(nc, in_maps, core_ids=list(range(NCORES)))
    return np.concatenate([r["out"] for r in res.results], axis=0)



# revision 26
# speedup vs baseline: 1.2668x; 1.2668x over previous
"""Trainium2 Bass kernel for nn_AutoEncoder_64854006170336.

Per-joint-embedding transformer encoder (B=1024, A=25 tokens, D=512, H=8, L=6).

Strategy:
- Data-parallel over batch: 8 cores x 128 batches each. No collectives.
- bf16 matmul operands, fp32 PSUM accumulation.
- Pack-contiguous token order: packs of G=4 batches; token (b=4g+j, pos a)
  lives at column 112*g + 25*j + a of the D-major tensors (112 = padded pack
  stride for DMA-transpose's mult-16 partition rule; cols 100..111 of each
  pack are dead).
- D-major tensors hT/kT/qT/oT [128, 4, 3584] (row d = ch*128 + p) feed all
  matmuls with contiguous stationary operands; token-major pack tiles
  [112, 512] carry residual/LayerNorm and are DMA-transposed back to D-major.
- q projection is weights-stationary (out = qT chunk directly); its moving
  operand is a strided position-gather AP, its evacuation a strided DVE write.
- Attention per (head, pack): scores via one 64x100x100 matmul, exp on ACT
  (scale=1/8 folds sqrt(depth)), cross-batch garbage zeroed by a mask
  multiply, softmax denominator via a ones-column matmul, normalization
  folded into the AV-psum evacuation.
- LayerNorm affine (g, b) folded host-side into next-layer projection
  weights/biases; residual state is xg = LNraw(p) * g (bf16) + biases carried
  in the matmuls (ACT per-partition bias or ones-row bias matmuls).
"""

from contextlib import ExitStack

import os

import numpy as np
import ml_dtypes

import concourse.bass as bass
import concourse.mybir as mybir
import concourse.tile as tile
from concourse import bacc
from concourse.bass_utils import run_bass_kernel_spmd

BF = ml_dtypes.bfloat16
F8 = ml_dtypes.float8_e4m3
QSC = 64.0
bf16 = mybir.dt.bfloat16
f32 = mybir.dt.float32
AF = mybir.ActivationFunctionType
ALU = mybir.AluOpType

B, J, DI, D, H, L = 1024, 24, 64, 512, 8, 6
A = J + 1            # 25 tokens
NCORES = 8
BC = B // NCORES     # 128 batches/core
G = 4                # batches per pack
NPACK = BC // G      # 32
PT = G * A           # 100 live tokens per pack
PTP = 112            # padded pack stride (mult of 16)
TW = NPACK * PTP     # 3584 D-major token columns
DEPTH = D // H       # 64
NCH = 4
LN_EPS = 1e-5
CAT = 4              # packs per score-psum concat
TOK_TILES = [(i * 512, 512) for i in range(7)]
CATS = [(g0, min(CAT, NPACK - g0)) for g0 in range(0, NPACK, CAT)]

_compiled = None
SIM_SAFE = os.environ.get("BASS_SIM", "0") == "1"


def _ap(tensor_ap, extra_offset, dims):
    return bass.AP(tensor=tensor_ap.tensor, offset=tensor_ap.offset + extra_offset,
                   ap=dims)


def _bcast_mid(ap, n):
    return bass.AP(tensor=ap.tensor, offset=ap.offset,
                   ap=[ap.ap[0], [0, n], *ap.ap[1:]])


def _bcast_last(ap, n):
    return bass.AP(tensor=ap.tensor, offset=ap.offset, ap=[*ap.ap, [0, n]])


class _Kern:
    def __init__(self):
        self.nc = bacc.Bacc(None, target_bir_lowering=False)
        nc = self.nc
        self.x_t = nc.dram_tensor("x_t", [DI, J, BC], bf16, kind="ExternalInput")
        self.cls_row = nc.dram_tensor("cls_row", [1, D], bf16, kind="ExternalInput")
        self.We_t = nc.dram_tensor("We_t", [J, DI, D], bf16, kind="ExternalInput")
        self.be_t = nc.dram_tensor("be_t", [J, D], bf16, kind="ExternalInput")
        self.Wk_t = nc.dram_tensor("Wk_t", [L, 128, NCH, D], bf16, kind="ExternalInput")
        self.Wv_t = nc.dram_tensor("Wv_t", [L, 128, NCH, D], bf16, kind="ExternalInput")
        self.Wo_t = nc.dram_tensor("Wo_t", [L, 128, NCH, D], bf16, kind="ExternalInput")
        self.f8 = mybir.dt.float8e4
        self.Wq_t = nc.dram_tensor(
            "Wq_t", [L, A, 128, NCH, D], self.f8, kind="ExternalInput"
        )
        self.bk_t = nc.dram_tensor("bk_t", [L, 128, NCH], f32, kind="ExternalInput")
        self.bq_t = nc.dram_tensor("bq_t", [L, A, 128, NCH], f32, kind="ExternalInput")
        self.bo_t = nc.dram_tensor("bo_t", [L, D], bf16, kind="ExternalInput")
        self.g_t = nc.dram_tensor("g_t", [L - 1, D], bf16, kind="ExternalInput")
        self.g5_row = nc.dram_tensor("g5_row", [1, D], f32, kind="ExternalInput")
        self.b5_row = nc.dram_tensor("b5_row", [1, D], f32, kind="ExternalInput")
        self.MU_t = nc.dram_tensor("MU_t", [G + 1, PT], bf16, kind="ExternalInput")
        self.MV_t = nc.dram_tensor("MV_t", [G + 1, CAT * PT], bf16, kind="ExternalInput")
        self.out_t = nc.dram_tensor("out", [BC, A, D], f32, kind="ExternalOutput")

    def build(self):
        nc = self.nc
        with ExitStack() as ctx:
            tc = ctx.enter_context(tile.TileContext(nc))
            p = lambda name, bufs, space="SBUF": ctx.enter_context(
                tc.tile_pool(name=name, bufs=bufs, space=space)
            )
            self.big = p("big", 1)
            self.xgp = p("xgp", NPACK)
            self.stage = p("stage", 2)
            self.vstore = p("vstore", 4)
            self.otokp = p("otok", 2)
            self.oTpool = p("oTpool", NPACK)
            self.xtp = p("xtp", 8)
            self.wts = p("wts", 1)
            self.wkp = p("wkp", 2)
            self.wqp = p("wqp", 2)
            self.rows = p("rows", 2)
            self.emws = p("emws", 2)
            self.stats = p("stats", 4)
            self.spool = p("spool", 3)
            self.ppool = p("ppool", 3)
            self.fpool = p("fpool", 2)
            self.edp = p("edram", 1, "DRAM")
            self.psA = p("psA", 2, "PSUM")
            self.psS = p("psS", 2, "PSUM")
            self.psV = p("psV", 2, "PSUM")
            self.psD = p("psD", 2, "PSUM")
            self._consts()
            self._embedding()
            for l in range(L):
                self._layer(l)
        nc.compile()
        return nc

    def _consts(self):
        nc, big = self.nc, self.big
        self.hT = big.tile([128, NCH, TW], bf16, tag="hT")
        self.kT = big.tile([128, NCH, TW], bf16, tag="kT")
        self.qT = big.tile([128, NCH, TW], bf16, tag="qT")
        self.oTp = [None] * NPACK
        self.MU = big.tile([G + 1, PT], bf16, tag="MU")
        self.MV = big.tile([G + 1, CAT * PT], bf16, tag="MV")
        self.ones_row = big.tile([1, 128], bf16, tag="ones_row")
        self.ones_col = big.tile([PT, 1], bf16, tag="ones_col")
        self.eps_t = big.tile([128, 1], f32, tag="eps")
        self.g5b = big.tile([128, D], f32, tag="g5b")
        self.b5b = big.tile([128, D], f32, tag="b5b")
        nc.scalar.dma_start(out=self.MU[:], in_=self.MU_t[:])
        nc.scalar.dma_start(out=self.MV[:], in_=self.MV_t[:])
        nc.vector.memset(self.ones_row[:], 1.0)
        nc.vector.memset(self.ones_col[:], 1.0)
        nc.vector.memset(self.eps_t[:], LN_EPS)
        nc.scalar.dma_start(out=self.g5b[:], in_=self.g5_row[:].to_broadcast((128, D)))
        nc.scalar.dma_start(out=self.b5b[:], in_=self.b5_row[:].to_broadcast((128, D)))

    def _pos_ap(self, big3, kc, a):
        """Moving-operand AP: [128, NPACK, G] = cols 112g + 25j + a of chunk kc."""
        base = big3[:, kc, :]
        return _ap(base, a, [base.ap[0], [PTP, NPACK], [A, G]])

    def _pos_out_ap(self, big3, a):
        """[128, NCH, NPACK, G] strided evac target across all chunks."""
        base = big3[:]
        return _ap(base, a, [base.ap[0], [TW, NCH], [PTP, NPACK], [A, G]])

    def _head_win(self, tens, h, g):
        return tens[(h % 2) * 64 : (h % 2) * 64 + 64, h // 2,
                    g * PTP : g * PTP + PT]

    def _embedding(self):
        nc = self.nc
        e_dram = self.edp.tile([BC, A, D], bf16, tag="edram")
        cls_sb = self.stage.tile([BC, D], bf16, tag="xhat", name="cls_sb")
        nc.scalar.dma_start(out=cls_sb[:], in_=self.cls_row[:].to_broadcast((BC, D)))
        nc.sync.dma_start(out=e_dram[:, 0, :], in_=cls_sb[:])
        for j in range(J):
            xj = self.emws.tile([DI, BC], bf16, tag="xj")
            nc.scalar.dma_start(out=xj[:], in_=self.x_t[:, j, :])
            wej = self.emws.tile([DI, D], bf16, tag="wej")
            nc.scalar.dma_start(out=wej[:], in_=self.We_t[j])
            bej = self.rows.tile([1, D], bf16, tag="brow")
            nc.scalar.dma_start(out=bej[:], in_=self.be_t[j : j + 1, :])
            ps = self.psA.tile([128, D], f32, tag="pp")
            nc.tensor.matmul(ps[:], xj[:], wej[:], start=True, stop=False)
            nc.tensor.matmul(ps[:], self.ones_row[:], bej[:], start=False, stop=True)
            ej = self.stage.tile([BC, D], bf16, tag="xhat", name="ej")
            nc.vector.tensor_copy(ej[:], ps[:])
            nc.sync.dma_start(out=e_dram[:, j + 1, :], in_=ej[:])
        self.xg = [None] * NPACK
        for g in range(NPACK):
            xg0 = self.xgp.tile([PTP, D], bf16, tag="xg")
            if SIM_SAFE:
                nc.vector.memset(xg0[96:PTP, :], 0.0)
            nc.sync.dma_start(out=xg0[:PT, :], in_=e_dram[G * g : G * g + G, :, :])
            nc.sync.dma_start(
                out=self.hT[:, :, g * PTP : (g + 1) * PTP], in_=xg0[:], transpose=True
            )
            self.xg[g] = xg0

    def _layer_weights(self, l):
        nc = self.nc
        wk = self.wkp.tile([128, NCH, D], bf16, tag="wk")
        nc.scalar.dma_start(out=wk[:], in_=self.Wk_t[l])
        wv = self.wts.tile([128, NCH, D], bf16, tag="wv")
        nc.scalar.dma_start(out=wv[:], in_=self.Wv_t[l])
        wo = self.wts.tile([128, NCH, D], bf16, tag="wo")
        nc.scalar.dma_start(out=wo[:], in_=self.Wo_t[l])
        bk = self.stats.tile([128, NCH], f32, tag="bk")
        nc.scalar.dma_start(out=bk[:], in_=self.bk_t[l])
        bo_row = self.rows.tile([1, D], bf16, tag="borow")
        nc.scalar.dma_start(out=bo_row[:], in_=self.bo_t[l : l + 1, :])
        glb = None
        if l < L - 1:
            glb = self.stage.tile([128, D], bf16, tag="glb")
            nc.gpsimd.dma_start(
                out=glb[:], in_=self.g_t[l : l + 1, :].to_broadcast((128, D))
            )
        return wk, wv, wo, bk, bo_row, glb

    def _kproj(self, wk, bk):
        nc = self.nc
        for oc in range(NCH):
            for (c0, cn) in TOK_TILES:
                ps = self.psA.tile([128, D], f32, tag="pp")
                for kc in range(NCH):
                    nc.tensor.matmul(
                        ps[:, :cn],
                        wk[:, kc, oc * 128 : (oc + 1) * 128],
                        self.hT[:, kc, c0 : c0 + cn],
                        start=(kc == 0),
                        stop=(kc == NCH - 1),
                    )
                nc.scalar.activation(
                    self.kT[:, oc, c0 : c0 + cn], ps[:, :cn],
                    AF.Identity, bias=bk[:, oc : oc + 1], scale=1.0,
                )

    def _qproj(self, l):
        nc = self.nc
        for a in range(A):
            wq = self.wqp.tile([128, NCH, D], self.f8, tag="wq")
            nc.scalar.dma_start(out=wq[:], in_=self.Wq_t[l, a])
            bq = self.stats.tile([128, NCH], f32, tag="bq")
            nc.scalar.dma_start(out=bq[:], in_=self.bq_t[l, a])
            ps = self.psA.tile([128, NCH, 128], f32, tag="pp")
            for oc in range(NCH):
                for kc in range(NCH):
                    nc.tensor.matmul(
                        ps[:, oc, :],
                        wq[:, kc, oc * 128 : (oc + 1) * 128],
                        self._pos_ap(self.hT, kc, a),
                        start=(kc == 0),
                        stop=(kc == NCH - 1),
                    )
            nc.vector.tensor_tensor(
                self._pos_out_ap(self.qT, a),
                ps[:],
                _bcast_last(bq[:], 128),
                ALU.add,
            )

    def _vproj(self, wv):
        nc = self.nc
        v_sb = [None] * NPACK
        for g in range(NPACK):
            ps = self.psA.tile([128, D], f32, tag="pp")
            for kc in range(NCH):
                nc.tensor.matmul(
                    ps[:PT, :],
                    self.hT[:, kc, g * PTP : g * PTP + PT],
                    wv[:, kc, :],
                    start=(kc == 0),
                    stop=(kc == NCH - 1),
                )
            vt = self.vstore.tile([PT, D], bf16, tag="v")
            nc.scalar.copy(vt[:], ps[:PT, :])
            v_sb[g] = vt
        return v_sb

    def _attention(self, v_sb):
        nc = self.nc
        for (g0, ng) in CATS:
            xts = []
            for hp in range(H // 2):
                sc2 = [self.psS.tile([PT, CAT * PT], f32, tag="sc", name=f"sc{u}") for u in range(2)]
                for u in range(2):
                    nc.tensor.matmul(
                        sc2[u][:, : ng * PT], self.MU[:], self.MV[:, : ng * PT],
                        start=True, stop=False,
                    )
                for i in range(ng):
                    for u in range(2):
                        h = 2 * hp + u
                        nc.tensor.matmul(
                            sc2[u][:, i * PT : (i + 1) * PT],
                            self._head_win(self.kT, h, g0 + i),
                            self._head_win(self.qT, h, g0 + i),
                            start=False, stop=(i == ng - 1),
                        )
                for u in range(2):
                    xt = self.xtp.tile([PT, CAT * PT], bf16, tag="xt")
                    nc.scalar.activation(
                        xt[:, : ng * PT], sc2[u][:, : ng * PT], AF.Exp,
                        scale=1.0 / (8.0 * QSC),
                    )
                    xts.append(xt)
            for i in range(ng):
                self._av(xts, v_sb, g0, i)

    def _av(self, xts, v_sb, g0, i):
        nc = self.nc
        g = g0 + i
        avps = self.psV.tile([PT, D], f32, tag="av")
        sps = self.psD.tile([PT, H], f32, tag="s")
        for h in range(H):
            xsl = xts[h][:, i * PT : (i + 1) * PT]
            nc.tensor.matmul(
                avps[:, h * DEPTH : (h + 1) * DEPTH],
                xsl, v_sb[g][:, h * DEPTH : (h + 1) * DEPTH],
                start=True, stop=True,
            )
            nc.tensor.matmul(
                sps[:, h : h + 1], xsl, self.ones_col[:], start=True, stop=True
            )
        rec = self.spool.tile([PT, H], f32, tag="rec")
        nc.vector.reciprocal(rec[:], sps[:])
        ot = self.otokp.tile([PTP, D], bf16, tag="otok")
        if SIM_SAFE:
            nc.vector.memset(ot[96:PTP, :], 0.0)
        nc.vector.tensor_tensor(
            ot[:PT, :].rearrange("p (h e) -> p h e", h=H),
            avps[:].rearrange("p (h e) -> p h e", h=H),
            _bcast_last(rec[:], DEPTH),
            ALU.mult,
        )
        oTg = self.oTpool.tile([128, NCH, PTP], bf16, tag="oT")
        nc.sync.dma_start(out=oTg[:], in_=ot[:], transpose=True)
        self.oTp[g] = oTg

    def _opack(self, lctx, g):
        l, wo, bo_row, glb = lctx
        nc = self.nc
        if True:
            ps = self.psA.tile([128, D], f32, tag="pp")
            for kc in range(NCH):
                nc.tensor.matmul(
                    ps[:PT, :],
                    self.oTp[g][:, kc, :PT],
                    wo[:, kc, :],
                    start=(kc == 0), stop=False,
                )
            nc.tensor.matmul(
                ps[:PT, :], self.ones_row[:, :PT], bo_row[:], start=False, stop=True
            )
            pt = self.ppool.tile([PT, D], f32, tag="p")
            nc.vector.tensor_add(pt[:], ps[:PT, :], self.xg[g][:PT, :])
            st6 = self.stats.tile([PT, 6], f32, tag="st6")
            nc.vector.bn_stats(st6[:], pt[:])
            mv = self.stats.tile([PT, 2], f32, tag="mv")
            nc.vector.bn_aggr(mv[:], st6[:])
            std = self.stats.tile([PT, 1], f32, tag="std")
            nc.scalar.activation(std[:], mv[:, 1:2], AF.Sqrt, bias=self.eps_t[:PT, :])
            rstd = self.stats.tile([PT, 1], f32, tag="rstd")
            nc.vector.reciprocal(rstd[:], std[:])
            nmr = self.stats.tile([PT, 1], f32, tag="nmr")
            nc.vector.tensor_scalar(
                nmr[:], mv[:, 0:1], rstd[:], -1.0, ALU.mult, ALU.mult
            )
            if l < L - 1:
                xh = self.stage.tile([PTP, D], bf16, tag="xhat")
                if SIM_SAFE:
                    nc.vector.memset(xh[96:PTP, :], 0.0)
                nc.scalar.activation(
                    xh[:PT, :], pt[:], AF.Identity, bias=nmr[:], scale=rstd[:]
                )
                nc.sync.dma_start(
                    out=self.hT[:, :, g * PTP : (g + 1) * PTP], in_=xh[:],
                    transpose=True,
                )
                xgn = self.xgp.tile([PTP, D], bf16, tag="xg")
                nc.gpsimd.tensor_mul(xgn[:PT, :], xh[:PT, :], glb[:PT, :])
                self.xg[g] = xgn
            else:
                of = self.fpool.tile([PT, D], f32, tag="of")
                nc.scalar.activation(
                    of[:], pt[:], AF.Identity, bias=nmr[:], scale=rstd[:]
                )
                nc.vector.tensor_mul(of[:], of[:], self.g5b[:PT, :])
                nc.vector.tensor_add(of[:], of[:], self.b5b[:PT, :])
                nc.sync.dma_start(
                    out=self.out_t[G * g : G * g + G, :, :], in_=of[:]
                )

    def _layer(self, l):
        wk, wv, wo, bk, bo_row, glb = self._layer_weights(l)
        self._kproj(wk, bk)
        self._qproj(l)
        v_sb = self._vproj(wv)
        self._attention(v_sb)
        for g in range(NPACK):
            self._opack((l, wo, bo_row, glb), g)


def _build():
    return _Kern().build()


def _prep_inputs(inputs):
    """Host-side fold + layout prep. Returns (shared dict, per-core x list)."""
    f = lambda v: np.asarray(v, dtype=np.float64)
    x = np.asarray(inputs["x"], dtype=np.float32)
    We, be = f(inputs["We"]), f(inputs["be"])
    cls_token = f(inputs["cls_token"])
    Wk, bk = f(inputs["Wk"]), f(inputs["bk"])
    Wv, bv = f(inputs["Wv"]), f(inputs["bv"])
    Wq, bq = f(inputs["Wq"]), f(inputs["bq"])
    Wo, bo = f(inputs["Wo"]), f(inputs["bo"])
    ln_g, ln_b = f(inputs["ln_g"]), f(inputs["ln_b"])

    def chunk_w(w):  # [512, 512] -> [128, 4, 512]
        return np.ascontiguousarray(
            w.reshape(NCH, 128, D).transpose(1, 0, 2)
        ).astype(BF)

    def chunk_b(b):  # [512] -> [128, 4]
        return np.ascontiguousarray(b.reshape(NCH, 128).T).astype(np.float32)

    Wk_t = np.zeros((L, 128, NCH, D), BF)
    Wv_t = np.zeros((L, 128, NCH, D), BF)
    Wo_t = np.zeros((L, 128, NCH, D), BF)
    Wq_t = np.zeros((L, A, 128, NCH, D), F8)
    bk_t = np.zeros((L, 128, NCH), np.float32)
    bq_t = np.zeros((L, A, 128, NCH), np.float32)
    bo_t = np.zeros((L, D), BF)
    for l in range(L):
        gf = ln_g[l - 1] if l > 0 else np.ones(D)
        bf_ = ln_b[l - 1] if l > 0 else np.zeros(D)
        bv_f = bf_ @ Wv[l] + bv[l]
        Wk_t[l] = chunk_w(gf[:, None] * Wk[l])
        Wv_t[l] = chunk_w(gf[:, None] * Wv[l])
        Wo_t[l] = chunk_w(Wo[l])
        bk_t[l] = chunk_b(bf_ @ Wk[l] + bk[l])
        bo_t[l] = (bv_f @ Wo[l] + bo[l] + bf_).astype(BF)
        for a in range(A):
            Wq_t[l, a] = chunk_w(QSC * gf[:, None] * Wq[l, a]).astype(
                np.float32).astype(F8)
            bq_t[l, a] = chunk_b(QSC * (bf_ @ Wq[l, a] + bq[l, a]))

    MB = 400.0 * QSC  # exp(-MB/(8*QSC)) == 0 in bf16; diag terms cancel
    MU = np.zeros((G + 1, PT), BF)
    MV = np.zeros((G + 1, CAT * PT), BF)
    MU[0, :] = 1.0
    MV[0, :] = -MB
    for j in range(G):
        MU[1 + j, j * A : (j + 1) * A] = 1.0
        for i in range(CAT):
            MV[1 + j, i * PT + j * A : i * PT + (j + 1) * A] = MB

    shared = {
        "cls_row": cls_token.reshape(1, D).astype(BF),
        "We_t": We.astype(BF),
        "be_t": be.astype(BF),
        "Wk_t": Wk_t, "Wv_t": Wv_t, "Wo_t": Wo_t, "Wq_t": Wq_t,
        "bk_t": bk_t, "bq_t": bq_t, "bo_t": bo_t,
        "g_t": ln_g[: L - 1].astype(BF),
        "g5_row": ln_g[L - 1].reshape(1, D).astype(np.float32),
        "b5_row": ln_b[L - 1].reshape(1, D).astype(np.float32),
        "MU_t": MU, "MV_t": MV,
    }
    x_cores = []
    for c in range(NCORES):
        xc = x[c * BC : (c + 1) * BC]            # [128, 24, 64]
        x_cores.append(np.ascontiguousarray(xc.transpose(2, 1, 0)).astype(BF))
    return shared, x_cores


def kernel(**inputs) -> np.ndarray:
    global _compiled
    if _compiled is None:
        _compiled = _build()
    nc = _compiled
    shared, x_cores = _prep_inputs(inputs)
    in_maps = [{**shared, "x_t": x_cores[c]} for c in range(NCORES)]
    res = run_bass_kernel_spmd(nc, in_maps, core_ids=list(range(NCORES)))
    return np.concatenate([r["out"] for r in res.results], axis=0)

